# revision 15
# baseline (speedup 1.0000x reference)
"""NeRF-W MLP forward kernel for 8 Trainium2 NeuronCores.

Data-parallel: N=262144 points split 8 ways; weights + appearance table
replicated.  Per core the network runs feature-major ([feature, point]
layout) in 512-point tiles:

  assembly matmul (fp32)     -> posenc args 2^l*x (+pi/2 for cos rows)
  2x DVE mod passes          -> range-reduce args into [-pi, pi)
  ACT Sin                    -> sin/cos features
  8x (matmul fp32r + ReLU)   -> hidden layers (skip concat at layer 4)
  fused rgb1 matmul          -> rgb1_W[:, :128] @ feat_W precomputed on host
  dma_gather + DVE transpose -> appearance embeddings, feature-major stripes
  transposed sigma/rgb2 mms  -> outputs point-major for contiguous DMA out
"""

import math

import numpy as np

import concourse.bacc as bacc
import concourse.mybir as mybir
import concourse.tile as tile
from concourse.bass_utils import run_bass_kernel_spmd

F32 = mybir.dt.float32
F32R = mybir.dt.float32r
I16 = mybir.dt.int16
AF = mybir.ActivationFunctionType
ALU = mybir.AluOpType

N_CORES = 8
NT = 512          # points per tile
GCH = 2048        # points per dma_gather call
POS_FREQS = 10
DIR_FREQS = 4
HIDDEN = 128
APP_DIM = 32
NUM_IMAGES = 1000

TWO_PI = float(np.float32(2.0 * math.pi))
MAGIC = float(np.float32(1.5 * 2.0 ** 23))  # float round-to-nearest-int trick

# args rows: 0-2 x, 3-62 x sincos, 63 pad, 64-66 d, 67-90 d sincos
ENC_ROWS = 91
# enc tile additionally holds the appearance embedding at rows 91-122
ENC_FULL = 123

LAST_RESULTS = None


def _r(ap):
    return ap.bitcast(F32R)


def _relu_to(nc, dst, src, bias_ap, on_act):
    if on_act:
        nc.scalar.activation(dst, src, AF.Relu, bias=bias_ap)
    else:
        nc.vector.tensor_scalar(dst, src, bias_ap, 0.0, op0=ALU.add, op1=ALU.max)


def build_program(n, use_f32r=True, app_mode="full"):
    """Build the per-core SPMD program for n points."""
    assert n % GCH == 0
    T = n // NT
    nc = bacc.Bacc("TRN2", target_bir_lowering=False, debug=False)

    FR = F32R if use_f32r else F32
    r = lambda ap: ap  # dtype is carried by the tiles themselves

    # ---- DRAM I/O ----
    xd_d = nc.dram_tensor("xd", [7, n], F32, kind="ExternalInput")
    idx_d = nc.dram_tensor("idxw", [128, n // 16], I16, kind="ExternalInput")
    app_d = nc.dram_tensor("appt", [NUM_IMAGES, 64], F32, kind="ExternalInput")
    xdr_d = nc.dram_tensor("xdr", [6, n], FR, kind="ExternalInput")
    rs_d = nc.dram_tensor("rs", [7, ENC_ROWS], F32, kind="ExternalInput")
    w0_d = nc.dram_tensor("w0", [63, 128], FR, kind="ExternalInput")
    wh_d = nc.dram_tensor("wh", [128, 7, 128], FR, kind="ExternalInput")
    w4e_d = nc.dram_tensor("w4e", [63, 128], FR, kind="ExternalInput")
    b_d = nc.dram_tensor("b", [128, 8], F32, kind="ExternalInput")
    wf_d = nc.dram_tensor("wf", [128, 64], FR, kind="ExternalInput")
    bf_d = nc.dram_tensor("bf", [64, 1], F32, kind="ExternalInput")
    wde_d = nc.dram_tensor("wde", [27, 64], FR, kind="ExternalInput")
    wapp_d = nc.dram_tensor("wapp", [128, 64], FR, kind="ExternalInput")
    sigw_d = nc.dram_tensor("sigw", [128, 2], FR, kind="ExternalInput")
    sigb_d = nc.dram_tensor("sigb", [128, 1], F32, kind="ExternalInput")
    w2e_d = nc.dram_tensor("w2e", [65, 4], FR, kind="ExternalInput")
    ones_d = nc.dram_tensor("ones", [1, NT], FR, kind="ExternalInput")
    appx_d = nc.dram_tensor("appx", [32, n], FR, kind="ExternalInput")

    rgb_o = nc.dram_tensor("rgb", [128, T * 12], F32, kind="ExternalOutput")
    sig_o = nc.dram_tensor("sig", [128, T * 4], F32, kind="ExternalOutput")

    with tile.TileContext(nc) as tc:
        with (
            tc.tile_pool(name="const", bufs=1) as const,
            tc.tile_pool(name="io", bufs=4) as io,
            tc.tile_pool(name="encp", bufs=3) as encp,
            tc.tile_pool(name="actp", bufs=5) as actp,
            tc.tile_pool(name="happ", bufs=2) as happ,
            tc.tile_pool(name="rndp", bufs=2) as rndp,
            tc.tile_pool(name="ps_args", bufs=2, space="PSUM") as ps_args,
            tc.tile_pool(name="ps_h", bufs=3, space="PSUM") as ps_h,
            tc.tile_pool(name="ps_rgb", bufs=1, space="PSUM") as ps_rgb,
            tc.tile_pool(name="ps_out", bufs=2, space="PSUM") as ps_out,
        ):
            # ---- resident constants ----
            rs_t = const.tile([7, ENC_ROWS], F32)
            nc.sync.dma_start(rs_t[:, :], rs_d[:, :])
            w0_t = const.tile([63, 128], FR)
            nc.sync.dma_start(w0_t[:, :], w0_d[:, :])
            wh_t = const.tile([128, 7, 128], FR)
            nc.sync.dma_start(wh_t[:, :, :], wh_d[:, :, :])
            w4e_t = const.tile([63, 128], FR)
            nc.sync.dma_start(w4e_t[:, :], w4e_d[:, :])
            b_t = const.tile([128, 8], F32)
            nc.sync.dma_start(b_t[:, :], b_d[:, :])
            wf_t = const.tile([128, 64], FR)
            nc.sync.dma_start(wf_t[:, :], wf_d[:, :])
            bf_t = const.tile([64, 1], F32)
            nc.sync.dma_start(bf_t[:, :], bf_d[:, :])
            wde_t = const.tile([ENC_FULL, 64], FR)
            nc.sync.dma_start(wde_t[64:ENC_ROWS, :], wde_d[:, :])
            nc.sync.dma_start(wde_t[ENC_ROWS:ENC_FULL, :], wapp_d[0:32, :])
            sigw_t = const.tile([128, 2], FR)
            nc.sync.dma_start(sigw_t[:, :], sigw_d[:, :])
            sigb_t = const.tile([128, 1], F32)
            nc.sync.dma_start(sigb_t[:, :], sigb_d[:, :])
            w2e_t = const.tile([65, 4], FR)
            nc.sync.dma_start(w2e_t[:, :], w2e_d[:, :])
            sig_out = const.tile([128, T * 4], F32)
            rgb_out = const.tile([128, T * 12], F32)

            # persistent [65, NT] tiles: row 64 is the constant-1 row used to
            # fold the rgb2 bias into its matmul
            hc_ring = [
                const.tile([65, NT], FR, tag=f"hc{i}", name=f"hc{i}")
                for i in range(4)
            ]
            for hct in hc_ring:
                nc.sync.dma_start(hct[64:65, :], ones_d[:, :])

            appT = None
            for t in range(T):
                tsl = slice(t * NT, (t + 1) * NT)

                # ---- positional encoding ----
                xd_t = io.tile([7, NT], F32)
                nc.sync.dma_start(xd_t[:, :], xd_d[:, tsl])
                args = ps_args.tile([ENC_ROWS, NT], F32)
                # full-fp32 matmul: 2^l * x must not lose mantissa bits
                nc.tensor.matmul(args[:, :], rs_t[:, :], xd_t[:, :])
                # args = (2^l x + phase)/2pi; subtract the nearest integer
                # (magic-number rounding: fp32 add/sub, mode-independent)
                rnd = rndp.tile([ENC_ROWS, NT], F32)
                nc.vector.tensor_scalar(
                    rnd[:, :], args[:, :],
                    MAGIC, MAGIC, op0=ALU.add, op1=ALU.subtract,
                )
                nc.vector.tensor_tensor(args[:, :], args[:, :], rnd[:, :],
                                        ALU.subtract)
                enc = encp.tile([ENC_FULL, NT], FR)
                # rows 0-2/63-66 hold sin(0)=0; the passthrough DMAs below
                # overwrite the x/d rows afterwards
                nc.scalar.activation(enc[0:ENC_ROWS, :], args[:, :], AF.Sin, scale=TWO_PI)
                # passthrough rows (overwrite the garbage Sin wrote at 63:66)
                nc.sync.dma_start(enc[0:3, :], xdr_d[0:3, tsl])
                nc.sync.dma_start(enc[64:67, :], xdr_d[3:6, tsl])
                nc.sync.dma_start(enc[ENC_ROWS:ENC_FULL, :], appx_d[:, tsl])

                # ---- hidden layers ----
                h_prev = None
                h7 = None
                for i in range(8):
                    hps = ps_h.tile([128, NT], F32, tag="hps")
                    if i == 0:
                        nc.tensor.matmul(hps[:, :], r(w0_t[:, :]), r(enc[0:63, :]))
                    elif i == 4:
                        nc.tensor.matmul(
                            hps[:, :], r(wh_t[:, 3, :]), r(h_prev[:, :]),
                            start=True, stop=False,
                        )
                        nc.tensor.matmul(
                            hps[:, :], r(w4e_t[:, :]), r(enc[0:63, :]),
                            start=False, stop=True,
                        )
                    else:
                        wi = i - 1 if i < 4 else i - 1  # wh layers 1..7 -> idx 0..6
                        nc.tensor.matmul(hps[:, :], r(wh_t[:, wi, :]), r(h_prev[:, :]))
                    h = actp.tile([128, NT], FR, tag="h")
                    _relu_to(nc, h[:, :], hps[:, :], b_t[:, i : i + 1],
                             on_act=(i % 2 == 0))
                    h_prev = h
                h7 = h_prev

                # ---- rgb1 (feat layer folded into wf on host) ----
                # host permutes point order so app stripe i covers device
                # columns [128i, 128i+128) of the tile
                rgbps = ps_rgb.tile([64, NT], F32)
                nc.tensor.matmul(rgbps[:, :], r(wf_t[:, :]), r(h7[:, :]),
                                 start=True, stop=False)
                nc.tensor.matmul(rgbps[:, :], r(wde_t[64:ENC_FULL, :]),
                                 r(enc[64:ENC_FULL, :]),
                                 start=False, stop=True)

                # ---- sigma head (transposed: points land on partitions;
                # fp32r needs an even moving dim, so N=2 with a zero col) ----
                so = ps_out.tile([128, 24], F32)
                for c in range(4):
                    nc.tensor.matmul(
                        so[:, 2 * c : 2 * c + 2],
                        r(h7[:, 128 * c : 128 * (c + 1)]),
                        r(sigw_t[:, :]),
                    )

                # ---- rgb1 relu -> hc (row 64 stays 1.0) ----
                hct = hc_ring[t % 4]
                nc.scalar.activation(hct[0:64, :], rgbps[:, :], AF.Relu,
                                     bias=bf_t[:, 0:1])

                # ---- rgb2 head (transposed, N=4 with a zero col) ----
                for c in range(4):
                    nc.tensor.matmul(
                        so[:, 8 + 4 * c : 12 + 4 * c],
                        r(hct[:, 128 * c : 128 * (c + 1)]),
                        r(w2e_t[:, :]),
                    )

                # ---- output activations into staging tiles ----
                sig_in = so[:, 0:8].rearrange("p (c k) -> p c k", k=2)[:, :, 0]
                nc.scalar.activation(
                    sig_out[:, 4 * t : 4 * (t + 1)], sig_in, AF.Relu,
                    bias=sigb_t[:, 0:1],
                )
                rgb_in = so[:, 8:24].rearrange("p (c k) -> p c k", k=4)[:, :, 0:3]
                rgb_o_view = rgb_out[:, 12 * t : 12 * (t + 1)].rearrange(
                    "p (c k) -> p c k", k=3)
                nc.scalar.activation(rgb_o_view, rgb_in, AF.Sigmoid)

            nc.sync.dma_start(sig_o[:, :], sig_out[:, :])
            nc.sync.dma_start(rgb_o[:, :], rgb_out[:, :])

    nc.compile()
    return nc


def _host_prep(x, d, img_idx, pts_params, sigma_W, sigma_b, feat_W, feat_b,
               rgb1_W, rgb1_b, rgb2_W, rgb2_b, app_table):
    """Marshal full inputs into per-core in_maps (pure data movement +
    weight re-layout / exact weight algebra)."""
    N = x.shape[0]
    n = N // N_CORES

    f32 = np.float32
    x = np.asarray(x, f32)
    d = np.asarray(d, f32)
    img_idx = np.asarray(img_idx)

    # ---- shared (replicated) tensors ----
    app_pad = np.zeros((NUM_IMAGES, 64), f32)
    app_pad[:, :APP_DIM] = np.asarray(app_table, f32)

    # assembly matrix: args[m] = sum_k rs[k, m] * xd[k],
    # xd rows = [x0 x1 x2 d0 d1 d2 1]
    # args rows hold (2^l * v + phase) / 2pi; the kernel reduces mod 1 and
    # applies Sin with scale=2pi
    rs = np.zeros((7, ENC_ROWS), f32)
    inv2pi = 1.0 / (2.0 * math.pi)
    for l in range(POS_FREQS):
        for s in range(2):           # 0 = sin, 1 = cos
            for k in range(3):
                m = 3 + 6 * l + 3 * s + k
                rs[k, m] = np.float32(2.0 ** l * inv2pi)
                if s == 1:
                    rs[6, m] = np.float32(0.25)
    for l in range(DIR_FREQS):
        for s in range(2):
            for k in range(3):
                m = 67 + 6 * l + 3 * s + k
                rs[3 + k, m] = np.float32(2.0 ** l * inv2pi)
                if s == 1:
                    rs[6, m] = np.float32(0.25)

    Ws = [np.asarray(W, f32) for W, _ in pts_params]
    bs = [np.asarray(b, f32) for _, b in pts_params]
    w0 = np.ascontiguousarray(Ws[0].T)                      # [63, 128]
    wh = np.zeros((128, 7, 128), f32)                       # layers 1..7 lhsT
    for i in range(1, 8):
        Wi = Ws[i][:, :128] if i == 4 else Ws[i]
        wh[:, i - 1, :] = Wi.T
    w4e = np.ascontiguousarray(Ws[4][:, 128:191].T)         # [63, 128]
    b_all = np.stack(bs, axis=1).astype(f32)                # [128, 8]

    rgb1_W = np.asarray(rgb1_W, f32)
    rgb1_b = np.asarray(rgb1_b, f32)
    feat_W = np.asarray(feat_W, f32)
    feat_b = np.asarray(feat_b, f32)
    W1_feat = rgb1_W[:, :128]                               # [64, 128]
    wf = np.ascontiguousarray((W1_feat @ feat_W).T)         # [128, 64]
    bf = (W1_feat @ feat_b + rgb1_b).reshape(64, 1).astype(f32)
    wde = np.ascontiguousarray(rgb1_W[:, 128:155].T)        # [27, 64]
    wapp1 = np.ascontiguousarray(rgb1_W[:, 155:187].T)      # [32, 64]
    wapp = np.tile(wapp1, (4, 1))                           # [128, 64]

    sigw = np.zeros((128, 2), f32)
    sigw[:, 0] = np.asarray(sigma_W, f32).reshape(128)
    sigb = np.full((128, 1), np.asarray(sigma_b, f32).reshape(()), f32)
    w2e = np.zeros((65, 4), f32)
    w2e[0:64, 0:3] = np.asarray(rgb2_W, f32).T
    w2e[64, 0:3] = np.asarray(rgb2_b, f32)

    ones = np.ones((1, NT), f32)
    shared = dict(appt=app_pad, rs=rs, w0=w0, wh=wh, w4e=w4e, b=b_all, wf=wf, ones=ones,
                  bf=bf, wde=wde, wapp=wapp, sigw=sigw, sigb=sigb, w2e=w2e)

    # device column lam of a tile holds original point perm(lam):
    # perm(lam) = 128*c' + 32*i + j with i=lam//128, c'=(lam%128)//32, j=lam%32
    lam = np.arange(NT)
    perm_tile = 128 * ((lam % 128) // 32) + 32 * (lam // 128) + (lam % 32)
    ar = np.arange(n)
    perm_full = (ar // NT) * NT + perm_tile[ar % NT]

    in_maps = []
    for c in range(N_CORES):
        sl = slice(c * n, (c + 1) * n)
        xd = np.empty((7, n), f32)
        xd[0:3] = x[sl].T[:, perm_full]
        xd[3:6] = d[sl].T[:, perm_full]
        xd[6] = 1.0
        xdr = np.ascontiguousarray(xd[0:6])
        idx_core = np.asarray(img_idx[sl], np.int16)
        idxw = np.tile(idx_core.reshape(n // 16, 16).T, (8, 1))  # [128, n//16]
        appx = np.ascontiguousarray(
            app_pad[np.asarray(img_idx[sl])[perm_full], :APP_DIM].T)
        in_maps.append(dict(xd=xd, xdr=xdr, idxw=np.ascontiguousarray(idxw),
                            appx=appx, **shared))
    return in_maps, n


def _unshard(results, n):
    """Invert the device output layout back to point order."""
    T = n // NT
    k = np.arange(n)
    t_ = k // NT
    cp_ = (k % NT) // 128       # c' in the original ordering
    i_ = (k % 128) // 32        # stripe -> device chunk
    j_ = k % 32
    p_ = 32 * cp_ + j_          # device partition
    rgbs, sigs = [], []
    for res in results:
        rgb_dev = np.asarray(res["rgb"]).reshape(128, T, 4, 3)
        sig_dev = np.asarray(res["sig"]).reshape(128, T, 4)
        rgbs.append(rgb_dev[p_, t_, i_, :])
        sigs.append(sig_dev[p_, t_, i_])
    rgb = np.concatenate(rgbs, axis=0)
    sigma = np.concatenate(sigs, axis=0)[:, None]
    return rgb.astype(np.float32), sigma.astype(np.float32)


def kernel(**inputs):
    global LAST_RESULTS
    import os
    app_mode = os.environ.get("KERNEL_APP_MODE", "full")
    in_maps, n = _host_prep(**inputs)
    nc = build_program(n, app_mode=app_mode)
    res = run_bass_kernel_spmd(nc, in_maps, core_ids=list(range(N_CORES)))
    LAST_RESULTS = res
    return _unshard(res.results, n)
